# revision 19
# baseline (speedup 1.0000x reference)
"""GAT message-passing kernel for Trainium2 (Bass/Tile), 8-core data parallel.

Problem: nn_GAT1 — per batch b:
    h = x @ W_pre                                   [N, U]
    e_s = h @ a_snd ; e_r = h @ a_rec               [N]
    logits[s, r] = leaky_relu(e_s[s] + e_r[r], 0.2)
    att = softmax over senders s (edges only, adj + self-loops)
    out[s, u] = sum_r att[s, r] * h[r, u]

Sharding: data-parallel over batch (B=8 -> one batch per NeuronCore).

Device layout is receiver-major ("transposed", r on partitions, s on free):
  - host passes xT (x transposed) and adjb[r, s] = (adjT - 1) * 1e9, i.e. an
    additive mask: 0 on edges (incl. self-loops), -1e9 on non-edges.
  - adjb is cast fp32->bf16 during the DMA itself (SWDGE cast), so the HBM
    read is still the full fp32 adjacency.
  - logitsT[r, s] = e_s[s] (broadcast matrix E) + e_r[r] (per-partition bias),
    leaky-relu via Prelu on ScalarE or TS/TS/max on VectorE.
  - masked logits = logitsT + adjb  (one 2x tensor_tensor add)
  - pmT = Exp(masked) on ScalarE, whose accum_out gives the softmax
    denominator for free.
  - outT[u, s] = sum_r (h[r, u] / den[r]) * pmT[r, s]  (PE, weight-stationary)
Host transposes outT back when gathering.
"""
import os
import sys

sys.path.insert(0, "/opt/trn_rl_repo")
sys.path.insert(0, "/opt/trn_rl_repo/concourse")

import numpy as np

import concourse.bass as bass
import concourse.bacc as bacc
import concourse.tile as tile
from concourse import mybir
from concourse.bass_utils import run_bass_kernel_spmd

B, N, F, U = 8, 2048, 128, 128
P = 128
NT = N // P          # 16 row tiles
ALPHA = 0.2          # leaky-relu slope
BIG = 1.0e9

K_ACT = int(os.environ.get("GAT_K_ACT", "4"))
# adjb DMA chunk sizes (r-tiles per SWDGE DMA); small first chunks for fast ramp
CHUNKS = [int(c) for c in os.environ.get("GAT_CHUNKS", "1,1,2,4,4,4").split(",")]
Z_ENG = os.environ.get("GAT_Z_ENG", "gpsimd")           # "vector" | "gpsimd"

f32 = mybir.dt.float32
f32r = mybir.dt.float32r
bf16 = mybir.dt.bfloat16
AF = mybir.ActivationFunctionType
OP = mybir.AluOpType

_cache = {}


def _build_nc():
    nc = bacc.Bacc("TRN2", target_bir_lowering=False, debug=False,
                   enable_asserts=False, num_devices=B)

    xt_d = nc.dram_tensor("xt", [F, N], f32r, kind="ExternalInput").ap()
    adjb_d = nc.dram_tensor("adjb", [N, N], f32, kind="ExternalInput").ap()
    w_d = nc.dram_tensor("w", [F, U], f32, kind="ExternalInput").ap()
    asnd_d = nc.dram_tensor("asnd", [U, 1], f32, kind="ExternalInput").ap()
    arec_d = nc.dram_tensor("arec", [U, 1], f32, kind="ExternalInput").ap()
    eye_d = nc.dram_tensor("eye", [P, P], f32, kind="ExternalInput").ap()
    outT_d = nc.dram_tensor("outT", [U, N], f32, kind="ExternalOutput").ap()

    with tile.TileContext(nc) as tc:
        with (
            tc.tile_pool(name="const", bufs=1) as const,
            tc.tile_pool(name="setup", bufs=2) as setup,
            tc.tile_pool(name="spsum", bufs=2, space="PSUM") as spsum,
            tc.tile_pool(name="adjp", bufs=3) as adjp,
            tc.tile_pool(name="work", bufs=3) as work,
            tc.tile_pool(name="mpsum", bufs=1, space="PSUM") as mpsum,
        ):
            # ---------------- constants ----------------
            w_sb = const.tile([F, U], f32)
            nc.sync.dma_start(out=w_sb[:], in_=w_d)
            asnd_f = const.tile([U, 1], f32)
            nc.sync.dma_start(out=asnd_f[:], in_=asnd_d)
            arec_f = const.tile([U, 1], f32)
            nc.sync.dma_start(out=arec_f[:], in_=arec_d)
            eye_sb = const.tile([P, P], f32)
            nc.sync.dma_start(out=eye_sb[:], in_=eye_d)
            ones_bf = const.tile([1, P], bf16)
            nc.vector.memset(ones_bf[:], 1.0)
            w_r = const.tile([F, U], f32r)
            nc.vector.tensor_copy(w_r[:], w_sb[:])

            # ---------------- xT load (host-transposed) ----------------
            xT_sb = const.tile([F, N], f32r)
            xt_dma = nc.sync.dma_start(out=xT_sb[:], in_=xt_d)
            xT_t = xT_sb.rearrange("f (t p) -> f t p", p=P)

            # ---------------- W^T, w_s, w_r vectors ----------------
            psw = spsum.tile([P, P], f32, tag="small")
            nc.tensor.transpose(psw[:], w_sb[:], eye_sb[:])
            wT_sb = setup.tile([U, F], f32)
            nc.any.tensor_copy(wT_sb[:], psw[:])

            ps_wsr = spsum.tile([P, 2], f32, tag="small")
            nc.tensor.matmul(ps_wsr[:, 0:1], lhsT=wT_sb[:], rhs=asnd_f[:],
                             start=True, stop=True)
            nc.tensor.matmul(ps_wsr[:, 1:2], lhsT=wT_sb[:], rhs=arec_f[:],
                             start=True, stop=True)
            wsr_sb = setup.tile([F, 2], f32r)
            nc.any.tensor_copy(wsr_sb[:], ps_wsr[:])

            # ---------------- e_s / e_r rows (f32r matmuls, N=512 chunks) -----
            es_row = setup.tile([1, N], bf16)
            er_row = setup.tile([1, N], f32)
            for c in range(4):
                xchunk = xT_sb[:, c * 512:(c + 1) * 512]
                ps_es = spsum.tile([1, 512], f32, tag="small")
                nc.tensor.matmul(ps_es[:], lhsT=wsr_sb[:, 0:1], rhs=xchunk,
                                 start=True, stop=True)
                nc.any.tensor_copy(es_row[:, c * 512:(c + 1) * 512], ps_es[:])
                ps_err = spsum.tile([1, 512], f32, tag="small")
                nc.tensor.matmul(ps_err[:], lhsT=wsr_sb[:, 1:2], rhs=xchunk,
                                 start=True, stop=True)
                nc.any.tensor_copy(er_row[:, c * 512:(c + 1) * 512], ps_err[:])

            # e_r as per-partition columns: 16 tiny [1,128]->[128,1] transposes
            ps_er = spsum.tile([P, NT], f32, tag="small")
            for j in range(NT):
                nc.tensor.transpose(ps_er[:, j:j + 1],
                                    er_row[:, j * P:(j + 1) * P],
                                    eye_sb[0:1, 0:1])
            er_sb = const.tile([P, NT], f32)
            nc.any.tensor_copy(er_sb[:], ps_er[:])
            er02_sb = const.tile([P, NT], f32)
            nc.vector.tensor_scalar(er02_sb[:], er_sb[:], ALPHA, None, op0=OP.mult)

            # ---------------- E = broadcast(e_s) (bf16) ----------------
            E_sb = const.tile([P, N], bf16)
            for c in range(4):
                ps_E = spsum.tile([P, 512], f32, tag="tp")
                nc.tensor.matmul(ps_E[:], lhsT=ones_bf[:],
                                 rhs=es_row[:, c * 512:(c + 1) * 512],
                                 start=True, stop=True)
                nc.any.tensor_copy(E_sb[:, c * 512:(c + 1) * 512], ps_E[:])

            # ---------------- h (bf16, natural layout) ----------------
            h_sb = const.tile([P, NT, U], bf16)
            for g in range(4):
                psh = spsum.tile([P, 512], f32, tag="tp")
                for k in range(4):
                    i = 4 * g + k
                    nc.tensor.matmul(psh[:, k * P:(k + 1) * P], lhsT=xT_t[:, i, :],
                                     rhs=w_r[:], start=True, stop=True)
                nc.scalar.copy(h_sb.rearrange("p t u -> p (t u)")[:, g * 512:(g + 1) * 512],
                               psh[:])

            # ---------------- main loop over r-tiles ----------------
            outT_ps = mpsum.tile([U, N], f32)   # 4 PSUM banks, accumulated over j
            # prefetch the whole adjb (8 MB bf16) with graduated chunk sizes so
            # the first tile lands fast and GpSimd finishes descriptor-gen early
            adjb_sb = const.tile([P, NT, N], bf16)
            assert sum(CHUNKS) == NT
            j0 = 0
            for g, csz in enumerate(CHUNKS):
                adj_dma = nc.gpsimd.dma_start(
                    out=adjb_sb[:, j0:j0 + csz, :],
                    in_=adjb_d[j0 * P:(j0 + csz) * P, :]
                    .rearrange("(c p) s -> p c s", p=P))
                if g == 0:
                    # keep the small setup DMAs ahead of the big adjb stream
                    bass._add_dep_helper(adj_dma.ins, xt_dma.ins, sync=True,
                                         reason="xt before adjb flood")
                j0 += csz
            for j in range(NT):
                if True:
                    a_j = work.tile([P, N], bf16, tag="a")
                    if (j * K_ACT) // NT != ((j + 1) * K_ACT) // NT:
                        # ACT-prelu tile (K_ACT of NT, spread out)
                        nc.scalar.activation(a_j[:], E_sb[:], AF.Prelu,
                                             bias=er_sb[:, j:j + 1], scale=1.0,
                                             alpha=ALPHA)
                    else:
                        z_j = work.tile([P, N], bf16, tag="z")
                        if Z_ENG == "gpsimd":
                            nc.gpsimd.tensor_scalar(z_j[:], E_sb[:],
                                                    er_sb[:, j:j + 1], None,
                                                    op0=OP.add)
                        else:
                            nc.vector.tensor_scalar(z_j[:], E_sb[:],
                                                    er_sb[:, j:j + 1], None,
                                                    op0=OP.add)
                        t_j = work.tile([P, N], bf16, tag="t")
                        nc.vector.tensor_scalar(t_j[:], E_sb[:], ALPHA,
                                                er02_sb[:, j:j + 1],
                                                op0=OP.mult, op1=OP.add)
                        nc.vector.tensor_max(a_j[:], z_j[:], t_j[:])
                    am_j = work.tile([P, N], bf16, tag="am")
                    nc.vector.tensor_add(am_j[:], a_j[:], adjb_sb[:, j, :])
                    pm_j = work.tile([P, N], bf16, tag="pm")
                    den_j = work.tile([P, 1], f32, tag="den")
                    nc.scalar.activation(pm_j[:], am_j[:], AF.Exp,
                                         accum_out=den_j[:])
                    inv_j = work.tile([P, 1], f32, tag="inv")
                    nc.vector.reciprocal(inv_j[:], den_j[:])
                    hp_j = work.tile([P, U], bf16, tag="hp")
                    nc.vector.tensor_scalar(hp_j[:], h_sb[:, j, :], inv_j[:], None,
                                            op0=OP.mult)
                    for c in range(4):
                        nc.tensor.matmul(outT_ps[:, c * 512:(c + 1) * 512],
                                         lhsT=hp_j[:],
                                         rhs=pm_j[:, c * 512:(c + 1) * 512],
                                         start=(j == 0), stop=(j == NT - 1))

            # ---------------- store ----------------
            outT_sb = setup.tile([U, N], f32)
            for c in range(4):
                nc.any.tensor_copy(outT_sb[:, c * 512:(c + 1) * 512],
                                   outT_ps[:, c * 512:(c + 1) * 512])
            nc.sync.dma_start(out=outT_d, in_=outT_sb[:])

    nc.compile()
    return nc


def kernel(x, adj, W_pre, a_snd, a_rec):
    """Full inputs in, full output out. Shards batch across 8 NeuronCores."""
    if "nc" not in _cache:
        _cache["nc"] = _build_nc()
    nc = _cache["nc"]

    x = np.asarray(x, dtype=np.float32)
    adj = np.asarray(adj, dtype=np.float32)
    W_pre = np.ascontiguousarray(np.asarray(W_pre, dtype=np.float32))
    a_snd = np.ascontiguousarray(np.asarray(a_snd, dtype=np.float32).reshape(U, 1))
    a_rec = np.ascontiguousarray(np.asarray(a_rec, dtype=np.float32).reshape(U, 1))

    # receiver-major additive mask: 0 on edges (+self-loops), -1e9 off edges
    adjb = np.ascontiguousarray(adj.transpose(0, 2, 1))
    idx = np.arange(N)
    adjb[:, idx, idx] = 1.0
    adjb -= 1.0
    adjb *= BIG

    xt = np.ascontiguousarray(x.transpose(0, 2, 1))   # [B, F, N]
    eye = np.eye(P, dtype=np.float32)
    in_maps = [
        {"xt": xt[b], "adjb": adjb[b], "w": W_pre, "asnd": a_snd, "arec": a_rec,
         "eye": eye}
        for b in range(B)
    ]
    trace = bool(int(os.environ.get("GAT_TRACE", "0")))
    res = run_bass_kernel_spmd(nc, in_maps, core_ids=list(range(B)), trace=trace,
                               trace_cores=list(range(B)) if trace else None)
    _cache["last_result"] = res
    out = np.stack([np.ascontiguousarray(r["outT"].T) for r in res.results])
    return out.astype(np.float32)


# revision 20
# speedup vs baseline: 1.0989x; 1.0989x over previous
"""GAT message-passing kernel for Trainium2 (Bass/Tile), 8-core data parallel.

Problem: nn_GAT1 — per batch b:
    h = x @ W_pre                                   [N, U]
    e_s = h @ a_snd ; e_r = h @ a_rec               [N]
    logits[s, r] = leaky_relu(e_s[s] + e_r[r], 0.2)
    att = softmax over senders s (edges only, adj + self-loops)
    out[s, u] = sum_r att[s, r] * h[r, u]

Sharding: data-parallel over batch (B=8 -> one batch per NeuronCore).

Device layout is receiver-major ("transposed", r on partitions, s on free):
  - host passes xT (x transposed) and adjb[r, s] = (adjT - 1) * 1e9, i.e. an
    additive mask: 0 on edges (incl. self-loops), -1e9 on non-edges.
  - adjb is cast fp32->bf16 during the DMA itself (SWDGE cast), so the HBM
    read is still the full fp32 adjacency.
  - logitsT[r, s] = e_s[s] (broadcast matrix E) + e_r[r] (per-partition bias),
    leaky-relu via Prelu on ScalarE or TS/TS/max on VectorE.
  - masked logits = logitsT + adjb  (one 2x tensor_tensor add)
  - pmT = Exp(masked) on ScalarE, whose accum_out gives the softmax
    denominator for free.
  - outT[u, s] = sum_r (h[r, u] / den[r]) * pmT[r, s]  (PE, weight-stationary)
Host transposes outT back when gathering.
"""
import os
import sys

sys.path.insert(0, "/opt/trn_rl_repo")
sys.path.insert(0, "/opt/trn_rl_repo/concourse")

import numpy as np

import concourse.bass as bass
import concourse.bacc as bacc
import concourse.tile as tile
from concourse import mybir
from concourse.bass_utils import run_bass_kernel_spmd

B, N, F, U = 8, 2048, 128, 128
P = 128
NT = N // P          # 16 row tiles
ALPHA = 0.2          # leaky-relu slope
BIG = 1.0e9

K_ACT = int(os.environ.get("GAT_K_ACT", "6"))
# adjb DMA chunk sizes (r-tiles per SWDGE DMA); small first chunks for fast ramp
CHUNKS = [int(c) for c in os.environ.get("GAT_CHUNKS", "1,1,2,4,4,4").split(",")]
Z_ENG = os.environ.get("GAT_Z_ENG", "vector")           # "vector" | "gpsimd"
# (gpsimd elementwise: measured 30us/op + it starves DVE via the shared port)

f32 = mybir.dt.float32
f32r = mybir.dt.float32r
bf16 = mybir.dt.bfloat16
AF = mybir.ActivationFunctionType
OP = mybir.AluOpType

_cache = {}


def _build_nc():
    nc = bacc.Bacc("TRN2", target_bir_lowering=False, debug=False,
                   enable_asserts=False, num_devices=B)

    xt_d = nc.dram_tensor("xt", [F, N], f32r, kind="ExternalInput").ap()
    adjb_d = nc.dram_tensor("adjb", [N, N], f32, kind="ExternalInput").ap()
    w_d = nc.dram_tensor("w", [F, U], f32, kind="ExternalInput").ap()
    asnd_d = nc.dram_tensor("asnd", [U, 1], f32, kind="ExternalInput").ap()
    arec_d = nc.dram_tensor("arec", [U, 1], f32, kind="ExternalInput").ap()
    eye_d = nc.dram_tensor("eye", [P, P], f32, kind="ExternalInput").ap()
    outT_d = nc.dram_tensor("outT", [U, N], f32, kind="ExternalOutput").ap()

    with tile.TileContext(nc) as tc:
        with (
            tc.tile_pool(name="const", bufs=1) as const,
            tc.tile_pool(name="setup", bufs=2) as setup,
            tc.tile_pool(name="spsum", bufs=2, space="PSUM") as spsum,
            tc.tile_pool(name="adjp", bufs=3) as adjp,
            tc.tile_pool(name="work", bufs=3) as work,
            tc.tile_pool(name="mpsum", bufs=1, space="PSUM") as mpsum,
        ):
            # ---------------- constants ----------------
            w_sb = const.tile([F, U], f32)
            nc.sync.dma_start(out=w_sb[:], in_=w_d)
            asnd_f = const.tile([U, 1], f32)
            nc.sync.dma_start(out=asnd_f[:], in_=asnd_d)
            arec_f = const.tile([U, 1], f32)
            nc.sync.dma_start(out=arec_f[:], in_=arec_d)
            eye_sb = const.tile([P, P], f32)
            nc.sync.dma_start(out=eye_sb[:], in_=eye_d)
            ones_bf = const.tile([1, P], bf16)
            nc.vector.memset(ones_bf[:], 1.0)
            w_r = const.tile([F, U], f32r)
            nc.vector.tensor_copy(w_r[:], w_sb[:])

            # ---------------- xT load (host-transposed) ----------------
            xT_sb = const.tile([F, N], f32r)
            xt_dma = nc.sync.dma_start(out=xT_sb[:], in_=xt_d)
            xT_t = xT_sb.rearrange("f (t p) -> f t p", p=P)

            # ---------------- W^T, w_s, w_r vectors ----------------
            psw = spsum.tile([P, P], f32, tag="small")
            nc.tensor.transpose(psw[:], w_sb[:], eye_sb[:])
            wT_sb = setup.tile([U, F], f32)
            nc.any.tensor_copy(wT_sb[:], psw[:])

            ps_wsr = spsum.tile([P, 2], f32, tag="small")
            nc.tensor.matmul(ps_wsr[:, 0:1], lhsT=wT_sb[:], rhs=asnd_f[:],
                             start=True, stop=True)
            nc.tensor.matmul(ps_wsr[:, 1:2], lhsT=wT_sb[:], rhs=arec_f[:],
                             start=True, stop=True)
            wsr_sb = setup.tile([F, 2], f32r)
            nc.any.tensor_copy(wsr_sb[:], ps_wsr[:])

            # ---------------- e_s / e_r rows (f32r matmuls, N=512 chunks) -----
            es_row = setup.tile([1, N], bf16)
            er_row = setup.tile([1, N], f32)
            for c in range(4):
                xchunk = xT_sb[:, c * 512:(c + 1) * 512]
                ps_es = spsum.tile([1, 512], f32, tag="small")
                nc.tensor.matmul(ps_es[:], lhsT=wsr_sb[:, 0:1], rhs=xchunk,
                                 start=True, stop=True)
                nc.any.tensor_copy(es_row[:, c * 512:(c + 1) * 512], ps_es[:])
                ps_err = spsum.tile([1, 512], f32, tag="small")
                nc.tensor.matmul(ps_err[:], lhsT=wsr_sb[:, 1:2], rhs=xchunk,
                                 start=True, stop=True)
                nc.any.tensor_copy(er_row[:, c * 512:(c + 1) * 512], ps_err[:])

            # e_r as per-partition columns: 16 tiny [1,128]->[128,1] transposes
            ps_er = spsum.tile([P, NT], f32, tag="small")
            for j in range(NT):
                nc.tensor.transpose(ps_er[:, j:j + 1],
                                    er_row[:, j * P:(j + 1) * P],
                                    eye_sb[0:1, 0:1])
            er_sb = const.tile([P, NT], f32)
            nc.any.tensor_copy(er_sb[:], ps_er[:])
            er02_sb = const.tile([P, NT], f32)
            nc.vector.tensor_scalar(er02_sb[:], er_sb[:], ALPHA, None, op0=OP.mult)

            # ---------------- E = broadcast(e_s) (bf16) ----------------
            E_sb = const.tile([P, N], bf16)
            for c in range(4):
                ps_E = spsum.tile([P, 512], f32, tag="tp")
                nc.tensor.matmul(ps_E[:], lhsT=ones_bf[:],
                                 rhs=es_row[:, c * 512:(c + 1) * 512],
                                 start=True, stop=True)
                nc.any.tensor_copy(E_sb[:, c * 512:(c + 1) * 512], ps_E[:])

            # ---------------- h (bf16, natural layout) ----------------
            h_sb = const.tile([P, NT, U], bf16)
            for g in range(4):
                psh = spsum.tile([P, 512], f32, tag="tp")
                for k in range(4):
                    i = 4 * g + k
                    nc.tensor.matmul(psh[:, k * P:(k + 1) * P], lhsT=xT_t[:, i, :],
                                     rhs=w_r[:], start=True, stop=True)
                nc.scalar.copy(h_sb.rearrange("p t u -> p (t u)")[:, g * 512:(g + 1) * 512],
                               psh[:])

            # ---------------- main loop over r-tiles ----------------
            outT_ps = mpsum.tile([U, N], f32)   # 4 PSUM banks, accumulated over j
            # prefetch the whole adjb (8 MB bf16) with graduated chunk sizes so
            # the first tile lands fast and GpSimd finishes descriptor-gen early
            adjb_sb = const.tile([P, NT, N], bf16)
            assert sum(CHUNKS) == NT
            j0 = 0
            for g, csz in enumerate(CHUNKS):
                adj_dma = nc.gpsimd.dma_start(
                    out=adjb_sb[:, j0:j0 + csz, :],
                    in_=adjb_d[j0 * P:(j0 + csz) * P, :]
                    .rearrange("(c p) s -> p c s", p=P))
                # keep the small setup DMAs ahead of the big adjb stream
                bass._add_dep_helper(adj_dma.ins, xt_dma.ins, sync=True,
                                     reason="xt before adjb flood")
                j0 += csz
            for j in range(NT):
                if True:
                    a_j = work.tile([P, N], bf16, tag="a")
                    if (j * K_ACT) // NT != ((j + 1) * K_ACT) // NT:
                        # ACT-prelu tile (K_ACT of NT, spread out)
                        nc.scalar.activation(a_j[:], E_sb[:], AF.Prelu,
                                             bias=er_sb[:, j:j + 1], scale=1.0,
                                             alpha=ALPHA)
                    else:
                        z_j = work.tile([P, N], bf16, tag="z")
                        if Z_ENG == "gpsimd":
                            nc.gpsimd.tensor_scalar(z_j[:], E_sb[:],
                                                    er_sb[:, j:j + 1], None,
                                                    op0=OP.add)
                        else:
                            nc.vector.tensor_scalar(z_j[:], E_sb[:],
                                                    er_sb[:, j:j + 1], None,
                                                    op0=OP.add)
                        t_j = work.tile([P, N], bf16, tag="t")
                        nc.vector.tensor_scalar(t_j[:], E_sb[:], ALPHA,
                                                er02_sb[:, j:j + 1],
                                                op0=OP.mult, op1=OP.add)
                        nc.vector.tensor_max(a_j[:], z_j[:], t_j[:])
                    am_j = work.tile([P, N], bf16, tag="am")
                    nc.vector.tensor_add(am_j[:], a_j[:], adjb_sb[:, j, :])
                    pm_j = work.tile([P, N], bf16, tag="pm")
                    den_j = work.tile([P, 1], f32, tag="den")
                    nc.scalar.activation(pm_j[:], am_j[:], AF.Exp,
                                         accum_out=den_j[:])
                    inv_j = work.tile([P, 1], f32, tag="inv")
                    nc.vector.reciprocal(inv_j[:], den_j[:])
                    hp_j = work.tile([P, U], bf16, tag="hp")
                    nc.vector.tensor_scalar(hp_j[:], h_sb[:, j, :], inv_j[:], None,
                                            op0=OP.mult)
                    for c in range(4):
                        nc.tensor.matmul(outT_ps[:, c * 512:(c + 1) * 512],
                                         lhsT=hp_j[:],
                                         rhs=pm_j[:, c * 512:(c + 1) * 512],
                                         start=(j == 0), stop=(j == NT - 1))

            # ---------------- store ----------------
            outT_sb = setup.tile([U, N], f32)
            for c in range(4):
                nc.any.tensor_copy(outT_sb[:, c * 512:(c + 1) * 512],
                                   outT_ps[:, c * 512:(c + 1) * 512])
            nc.sync.dma_start(out=outT_d, in_=outT_sb[:])

    nc.compile()
    return nc


def kernel(x, adj, W_pre, a_snd, a_rec):
    """Full inputs in, full output out. Shards batch across 8 NeuronCores."""
    if "nc" not in _cache:
        _cache["nc"] = _build_nc()
    nc = _cache["nc"]

    x = np.asarray(x, dtype=np.float32)
    adj = np.asarray(adj, dtype=np.float32)
    W_pre = np.ascontiguousarray(np.asarray(W_pre, dtype=np.float32))
    a_snd = np.ascontiguousarray(np.asarray(a_snd, dtype=np.float32).reshape(U, 1))
    a_rec = np.ascontiguousarray(np.asarray(a_rec, dtype=np.float32).reshape(U, 1))

    # receiver-major additive mask: 0 on edges (+self-loops), -1e9 off edges
    adjb = np.ascontiguousarray(adj.transpose(0, 2, 1))
    idx = np.arange(N)
    adjb[:, idx, idx] = 1.0
    adjb -= 1.0
    adjb *= BIG

    xt = np.ascontiguousarray(x.transpose(0, 2, 1))   # [B, F, N]
    eye = np.eye(P, dtype=np.float32)
    in_maps = [
        {"xt": xt[b], "adjb": adjb[b], "w": W_pre, "asnd": a_snd, "arec": a_rec,
         "eye": eye}
        for b in range(B)
    ]
    trace = bool(int(os.environ.get("GAT_TRACE", "0")))
    res = run_bass_kernel_spmd(nc, in_maps, core_ids=list(range(B)), trace=trace,
                               trace_cores=list(range(B)) if trace else None)
    _cache["last_result"] = res
    out = np.stack([np.ascontiguousarray(r["outT"].T) for r in res.results])
    return out.astype(np.float32)


# revision 21
# speedup vs baseline: 1.1422x; 1.0394x over previous
"""GAT message-passing kernel for Trainium2 (Bass/Tile), 8-core data parallel.

Problem: nn_GAT1 — per batch b:
    h = x @ W_pre                                   [N, U]
    e_s = h @ a_snd ; e_r = h @ a_rec               [N]
    logits[s, r] = leaky_relu(e_s[s] + e_r[r], 0.2)
    att = softmax over senders s (edges only, adj + self-loops)
    out[s, u] = sum_r att[s, r] * h[r, u]

Sharding: data-parallel over batch (B=8 -> one batch per NeuronCore).

Device layout is receiver-major ("transposed", r on partitions, s on free):
  - host passes xT (x transposed) and adjb[r, s] = (adjT - 1) * 1e9, i.e. an
    additive mask: 0 on edges (incl. self-loops), -1e9 on non-edges.
  - adjb is cast fp32->bf16 during the DMA itself (SWDGE cast), so the HBM
    read is still the full fp32 adjacency.
  - logitsT[r, s] = e_s[s] (broadcast matrix E) + e_r[r] (per-partition bias),
    leaky-relu via Prelu on ScalarE or TS/TS/max on VectorE.
  - masked logits = logitsT + adjb  (one 2x tensor_tensor add)
  - pmT = Exp(masked) on ScalarE, whose accum_out gives the softmax
    denominator for free.
  - outT[u, s] = sum_r (h[r, u] / den[r]) * pmT[r, s]  (PE, weight-stationary)
Host transposes outT back when gathering.
"""
import os
import sys

sys.path.insert(0, "/opt/trn_rl_repo")
sys.path.insert(0, "/opt/trn_rl_repo/concourse")

import numpy as np

import concourse.bass as bass
import concourse.bacc as bacc
import concourse.tile as tile
from concourse import mybir
from concourse.bass_utils import run_bass_kernel_spmd

B, N, F, U = 8, 2048, 128, 128
P = 128
NT = N // P          # 16 row tiles
ALPHA = 0.2          # leaky-relu slope
BIG = 1.0e9

K_ACT = int(os.environ.get("GAT_K_ACT", "6"))
# adjb DMA chunk sizes (r-tiles per SWDGE DMA); small first chunks for fast ramp
CHUNKS = [int(c) for c in os.environ.get("GAT_CHUNKS", "1,1,2,4,4,4").split(",")]
Z_ENG = os.environ.get("GAT_Z_ENG", "vector")           # "vector" | "gpsimd"
# (gpsimd elementwise: measured 30us/op + it starves DVE via the shared port)

f32 = mybir.dt.float32
f32r = mybir.dt.float32r
bf16 = mybir.dt.bfloat16
AF = mybir.ActivationFunctionType
OP = mybir.AluOpType

_cache = {}


def _build_nc():
    nc = bacc.Bacc("TRN2", target_bir_lowering=False, debug=False,
                   enable_asserts=False, num_devices=B)

    xt_d = nc.dram_tensor("xt", [F, N], f32r, kind="ExternalInput").ap()
    adjb_d = nc.dram_tensor("adjb", [N, N], f32, kind="ExternalInput").ap()
    w_d = nc.dram_tensor("w", [F, U], f32, kind="ExternalInput").ap()
    # wsr[:, 0] = W_pre @ a_snd, wsr[:, 1] = W_pre @ a_rec (host-derived params)
    wsr_d = nc.dram_tensor("wsr", [F, 2], f32r, kind="ExternalInput").ap()
    outT_d = nc.dram_tensor("outT", [U, N], f32, kind="ExternalOutput").ap()

    with tile.TileContext(nc) as tc:
        with (
            tc.tile_pool(name="const", bufs=1) as const,
            tc.tile_pool(name="setup", bufs=2) as setup,
            tc.tile_pool(name="spsum", bufs=2, space="PSUM") as spsum,
            tc.tile_pool(name="adjp", bufs=3) as adjp,
            tc.tile_pool(name="work", bufs=3) as work,
            tc.tile_pool(name="mpsum", bufs=1, space="PSUM") as mpsum,
        ):
            # ---------------- constants ----------------
            wsr_sb = const.tile([F, 2], f32r)
            nc.sync.dma_start(out=wsr_sb[:], in_=wsr_d)
            ones_bf = const.tile([1, P], bf16)
            nc.vector.memset(ones_bf[:], 1.0)
            one_f = const.tile([1, 1], f32)
            nc.vector.memset(one_f[:], 1.0)

            # ---------------- xT load (host-transposed) ----------------
            xT_sb = const.tile([F, N], f32r)
            xt_dma = nc.sync.dma_start(out=xT_sb[:], in_=xt_d)
            xT_t = xT_sb.rearrange("f (t p) -> f t p", p=P)

            w_sb = const.tile([F, U], f32)
            nc.sync.dma_start(out=w_sb[:], in_=w_d)

            # ---------------- e_s / e_r rows (f32r matmuls, N=512 chunks) -----
            es_row = setup.tile([1, N], bf16)
            er_row = setup.tile([1, N], f32)
            for c in range(4):
                xchunk = xT_sb[:, c * 512:(c + 1) * 512]
                ps_es = spsum.tile([1, 512], f32, tag="small")
                nc.tensor.matmul(ps_es[:], lhsT=wsr_sb[:, 0:1], rhs=xchunk,
                                 start=True, stop=True)
                nc.any.tensor_copy(es_row[:, c * 512:(c + 1) * 512], ps_es[:])
                ps_err = spsum.tile([1, 512], f32, tag="small")
                nc.tensor.matmul(ps_err[:], lhsT=wsr_sb[:, 1:2], rhs=xchunk,
                                 start=True, stop=True)
                nc.any.tensor_copy(er_row[:, c * 512:(c + 1) * 512], ps_err[:])

            # e_r as per-partition columns: 16 tiny [1,128]->[128,1] transposes
            ps_er = spsum.tile([P, NT], f32, tag="small")
            for j in range(NT):
                nc.tensor.transpose(ps_er[:, j:j + 1],
                                    er_row[:, j * P:(j + 1) * P],
                                    one_f[:])
            er_sb = const.tile([P, NT], f32)
            nc.any.tensor_copy(er_sb[:], ps_er[:])
            er02_sb = const.tile([P, NT], f32)
            nc.vector.tensor_scalar(er02_sb[:], er_sb[:], ALPHA, None, op0=OP.mult)

            # ---------------- E = broadcast(e_s) (bf16) ----------------
            E_sb = const.tile([P, N], bf16)
            for c in range(4):
                ps_E = spsum.tile([P, 512], f32, tag="tp")
                nc.tensor.matmul(ps_E[:], lhsT=ones_bf[:],
                                 rhs=es_row[:, c * 512:(c + 1) * 512],
                                 start=True, stop=True)
                nc.any.tensor_copy(E_sb[:, c * 512:(c + 1) * 512], ps_E[:])

            # ---------------- h (bf16, natural layout) ----------------
            w_r = const.tile([F, U], f32r)
            nc.vector.tensor_copy(w_r[:], w_sb[:])
            h_sb = const.tile([P, NT, U], bf16)
            for g in range(4):
                psh = spsum.tile([P, 512], f32, tag="tp")
                for k in range(4):
                    i = 4 * g + k
                    nc.tensor.matmul(psh[:, k * P:(k + 1) * P], lhsT=xT_t[:, i, :],
                                     rhs=w_r[:], start=True, stop=True)
                nc.scalar.copy(h_sb.rearrange("p t u -> p (t u)")[:, g * 512:(g + 1) * 512],
                               psh[:])

            # ---------------- main loop over r-tiles ----------------
            outT_ps = mpsum.tile([U, N], f32)   # 4 PSUM banks, accumulated over j
            # prefetch the whole adjb (8 MB bf16) with graduated chunk sizes so
            # the first tile lands fast and GpSimd finishes descriptor-gen early
            adjb_sb = const.tile([P, NT, N], bf16)
            assert sum(CHUNKS) == NT
            j0 = 0
            for g, csz in enumerate(CHUNKS):
                adj_dma = nc.gpsimd.dma_start(
                    out=adjb_sb[:, j0:j0 + csz, :],
                    in_=adjb_d[j0 * P:(j0 + csz) * P, :]
                    .rearrange("(c p) s -> p c s", p=P))
                # keep the small setup DMAs ahead of the big adjb stream
                bass._add_dep_helper(adj_dma.ins, xt_dma.ins, sync=True,
                                     reason="xt before adjb flood")
                j0 += csz
            for j in range(NT):
                if True:
                    a_j = work.tile([P, N], bf16, tag="a")
                    if (j * K_ACT) // NT != ((j + 1) * K_ACT) // NT:
                        # ACT-prelu tile (K_ACT of NT, spread out)
                        nc.scalar.activation(a_j[:], E_sb[:], AF.Prelu,
                                             bias=er_sb[:, j:j + 1], scale=1.0,
                                             alpha=ALPHA)
                    else:
                        z_j = work.tile([P, N], bf16, tag="z")
                        if Z_ENG == "gpsimd":
                            nc.gpsimd.tensor_scalar(z_j[:], E_sb[:],
                                                    er_sb[:, j:j + 1], None,
                                                    op0=OP.add)
                        else:
                            nc.vector.tensor_scalar(z_j[:], E_sb[:],
                                                    er_sb[:, j:j + 1], None,
                                                    op0=OP.add)
                        t_j = work.tile([P, N], bf16, tag="t")
                        nc.vector.tensor_scalar(t_j[:], E_sb[:], ALPHA,
                                                er02_sb[:, j:j + 1],
                                                op0=OP.mult, op1=OP.add)
                        nc.vector.tensor_max(a_j[:], z_j[:], t_j[:])
                    am_j = work.tile([P, N], bf16, tag="am")
                    nc.vector.tensor_add(am_j[:], a_j[:], adjb_sb[:, j, :])
                    pm_j = work.tile([P, N], bf16, tag="pm")
                    den_j = work.tile([P, 1], f32, tag="den")
                    nc.scalar.activation(pm_j[:], am_j[:], AF.Exp,
                                         accum_out=den_j[:])
                    inv_j = work.tile([P, 1], f32, tag="inv")
                    nc.vector.reciprocal(inv_j[:], den_j[:])
                    hp_j = work.tile([P, U], bf16, tag="hp")
                    nc.vector.tensor_scalar(hp_j[:], h_sb[:, j, :], inv_j[:], None,
                                            op0=OP.mult)
                    for c in range(4):
                        nc.tensor.matmul(outT_ps[:, c * 512:(c + 1) * 512],
                                         lhsT=hp_j[:],
                                         rhs=pm_j[:, c * 512:(c + 1) * 512],
                                         start=(j == 0), stop=(j == NT - 1))

            # ---------------- store ----------------
            outT_sb = setup.tile([U, N], f32)
            for c in range(4):
                nc.any.tensor_copy(outT_sb[:, c * 512:(c + 1) * 512],
                                   outT_ps[:, c * 512:(c + 1) * 512])
            nc.sync.dma_start(out=outT_d, in_=outT_sb[:])

    nc.compile()
    return nc


def kernel(x, adj, W_pre, a_snd, a_rec):
    """Full inputs in, full output out. Shards batch across 8 NeuronCores."""
    if "nc" not in _cache:
        _cache["nc"] = _build_nc()
    nc = _cache["nc"]

    x = np.asarray(x, dtype=np.float32)
    adj = np.asarray(adj, dtype=np.float32)
    W_pre = np.ascontiguousarray(np.asarray(W_pre, dtype=np.float32))
    a_snd = np.asarray(a_snd, dtype=np.float32).reshape(U)
    a_rec = np.asarray(a_rec, dtype=np.float32).reshape(U)
    wsr = np.ascontiguousarray(
        np.stack([W_pre @ a_snd, W_pre @ a_rec], axis=1).astype(np.float32))

    # receiver-major additive mask: 0 on edges (+self-loops), -1e9 off edges
    adjb = np.ascontiguousarray(adj.transpose(0, 2, 1))
    idx = np.arange(N)
    adjb[:, idx, idx] = 1.0
    adjb -= 1.0
    adjb *= BIG

    xt = np.ascontiguousarray(x.transpose(0, 2, 1))   # [B, F, N]
    in_maps = [
        {"xt": xt[b], "adjb": adjb[b], "w": W_pre, "wsr": wsr}
        for b in range(B)
    ]
    trace = bool(int(os.environ.get("GAT_TRACE", "0")))
    res = run_bass_kernel_spmd(nc, in_maps, core_ids=list(range(B)), trace=trace,
                               trace_cores=list(range(B)) if trace else None)
    _cache["last_result"] = res
    out = np.stack([np.ascontiguousarray(r["outT"].T) for r in res.results])
    return out.astype(np.float32)
